# revision 11
# baseline (speedup 1.0000x reference)
"""Trainium2 Bass kernel for batched multi-head attention with per-batch mask.

Reference semantics (B=8, H=16, S=1024, D=64):
    scores = Q @ K^T                      # [B,H,S,S]
    scores = where(mask[b], -1e9, scores) # per-batch mask
    attn   = softmax(scores / sqrt(D))
    out    = attn @ V

Key observations used here:
  * A masked batch (mask[b]=True) has constant scores, so softmax is exactly
    uniform (1/S) and out[b,h,q,:] = mean_k V[b,h,k,:].  That degenerate case
    is computed directly on host; only unmasked (b,h) units go to the device.
  * For unmasked units |scores/8| <= ~7, so softmax without max-subtraction
    (exp(x)/sum exp(x)) is numerically safe and mathematically identical.
  * Unmasked units are embarrassingly parallel; they are balanced across the
    8 NeuronCores (H=16 and B=8 make the unit count divisible by 8).

Device algorithm per unit (S=1024 split into 8 chunks of 128 along k):
  mm1:  T[k,q]  = sum_d K[k,d]*Q[q,d]       (lhsT = K^T chunk, rhs = Q^T)
        K=64 matmuls run at half rate, so chunk pairs are packed into row
        groups (0,0)/(64,0) with Q^T/K^T replicated on partitions 64..127 —
        the two half-array matmuls execute concurrently.
  exp:  E[k,q]  = exp(T/8)                  (ScalarE, PSUM->SBUF, fp32r out)
  mm2:  U[m,q] += sum_k Vx[k,m]*E[k,q]      (lhsT = Vx chunk = [V | ones | 0])
        Vx is zero-padded to 128 columns so M=128 stays on the fast path.
        -> U[0:64,q] = unnormalized out^T, U[64,q] = softmax denominator
  out:  recip denominator row, DMA-broadcast it across partitions, one DVE
        multiply, and write out^T [64, S]; the host transposes back.
"""

import numpy as np

B, H, S, D = 8, 16, 1024, 64
P = 128                      # SBUF partitions / k-chunk size
NCHUNK = S // P              # 8 k-chunks per unit
NHALF = 2                    # matmul moving operand is limited to N=512 fp32
NCORES = 8

_program_cache = {}


def _build_program(n_units):
    import concourse.bass as bass
    import concourse.mybir as mybir
    import concourse.tile as tile
    from concourse import bacc

    f32 = mybir.dt.float32
    f32r = mybir.dt.float32r
    nc = bacc.Bacc("TRN2", target_bir_lowering=False, debug=False)

    # qt/kt carry Q^T/K^T duplicated on partitions 64..127 (row-group packing)
    qt_d = nc.dram_tensor("qt", [n_units, P, S], f32r, kind="ExternalInput").ap()
    kt_d = nc.dram_tensor("kt", [n_units, P, S], f32r, kind="ExternalInput").ap()
    vx_d = nc.dram_tensor("vx", [n_units, S, P], f32r, kind="ExternalInput").ap()
    out_d = nc.dram_tensor("out", [n_units, D, S], f32, kind="ExternalOutput").ap()

    with tile.TileContext(nc) as tc:
        with (
            tc.tile_pool(name="qp", bufs=2) as qp,
            tc.tile_pool(name="kp", bufs=2) as kp,
            tc.tile_pool(name="vp", bufs=2) as vp,
            tc.tile_pool(name="ep", bufs=3) as ep,
            tc.tile_pool(name="rp", bufs=2) as rp,
            tc.tile_pool(name="bp", bufs=2) as bp,
            tc.tile_pool(name="op", bufs=2) as op,
            tc.tile_pool(name="pt", bufs=3, space="PSUM") as pt,   # 3 x 2 banks
            tc.tile_pool(name="pu", bufs=1, space="PSUM") as pu,   # 2 banks
        ):
            for j in range(n_units):
                qt = qp.tile([P, S], f32r)
                nc.sync.dma_start(qt, qt_d[j])
                kt = kp.tile([P, S], f32r)
                nc.sync.dma_start(kt, kt_d[j])
                vx = vp.tile([P, NCHUNK, P], f32r)
                nc.sync.dma_start(vx, vx_d[j].rearrange("(c p) d -> p c d", p=P))

                u_ps = pu.tile([P, S], f32)
                for cp in range(NCHUNK // 2):
                    ca, cb = 2 * cp, 2 * cp + 1
                    ta = pt.tile([P, S], f32, tag="tps")
                    tb = pt.tile([P, S], f32, tag="tps")
                    for h in range(NHALF):
                        qs = slice(h * 512, (h + 1) * 512)
                        nc.tensor.matmul(
                            ta[:, qs],
                            lhsT=kt[0:D, ca * P:(ca + 1) * P],
                            rhs=qt[0:D, qs],
                            start=True, stop=True,
                            tile_position=(0, 0),
                        )
                        nc.tensor.matmul(
                            tb[:, qs],
                            lhsT=kt[D:P, cb * P:(cb + 1) * P],
                            rhs=qt[D:P, qs],
                            start=True, stop=True,
                            tile_position=(64, 0),
                        )
                    for c, t_ps in ((ca, ta), (cb, tb)):
                        e_sb = ep.tile([P, S], f32r)
                        nc.scalar.activation(
                            e_sb, t_ps, mybir.ActivationFunctionType.Exp,
                            bias=0.0, scale=0.125,
                        )
                        for h in range(NHALF):
                            qs = slice(h * 512, (h + 1) * 512)
                            nc.tensor.matmul(
                                u_ps[:, qs],
                                lhsT=vx[:, c, :],
                                rhs=e_sb[:, qs],
                                start=(c == 0),
                                stop=(c == NCHUNK - 1),
                            )

                # normalize in [d, q] layout: reciprocal of the denominator
                # row (partition-aligned PSUM->SBUF), DMA-broadcast it across
                # partitions, then one full-width multiply.
                r65 = rp.tile([D + 1, S], f32)
                nc.vector.reciprocal(out=r65[D:D + 1, :], in_=u_ps[D:D + 1, :])
                r_bc = bp.tile([D, S], f32)
                r_src = r65[D:D + 1, :].unsqueeze(1).broadcast_to([1, D, S])
                nc.sync.dma_start(r_bc, r_src)
                o_sb = op.tile([D, S], f32)
                nc.vector.tensor_mul(out=o_sb, in0=u_ps[0:D, :], in1=r_bc)
                nc.sync.dma_start(out_d[j], o_sb)
    nc.compile()
    return nc


def _get_program(n_units):
    if n_units not in _program_cache:
        _program_cache[n_units] = _build_program(n_units)
    return _program_cache[n_units]


def _round_fp32r(x):
    """Round fp32 to the fp32r-representable set (bf16 hi + bf16 lo pair).

    The walrus verifier requires fp32r matmul operands to be pre-rounded;
    the PE's replicated fp32 path decomposes each value into two bf16s.
    """
    import ml_dtypes

    hi = x.astype(ml_dtypes.bfloat16).astype(np.float32)
    lo = (x - hi).astype(ml_dtypes.bfloat16).astype(np.float32)
    return hi + lo


def _prepare(Q, K, V, mask):
    """Host-side sharding. Returns (out_skeleton, units_per_core, in_maps)."""
    Q = np.ascontiguousarray(Q, dtype=np.float32)
    K = np.ascontiguousarray(K, dtype=np.float32)
    V = np.ascontiguousarray(V, dtype=np.float32)
    mask_b = np.asarray(mask).reshape(B).astype(bool)

    out = np.empty((B, H, S, D), dtype=np.float32)

    # Masked batches: softmax over a constant row is exactly uniform -> mean of V.
    for b in np.nonzero(mask_b)[0]:
        mv = V[b].mean(axis=1, dtype=np.float32)          # [H, D]
        out[b] = np.broadcast_to(mv[:, None, :], (H, S, D))

    units = [(b, h) for b in range(B) if not mask_b[b] for h in range(H)]
    if not units:
        return out, None, None

    # Pad to a multiple of NCORES with duplicates (identical redundant work).
    n_per = -(-len(units) // NCORES)
    padded = units + [units[0]] * (n_per * NCORES - len(units))
    per_core = [padded[i::NCORES] for i in range(NCORES)]

    QT = _round_fp32r(Q.transpose(0, 1, 3, 2))            # [B,H,D,S]
    KT = _round_fp32r(K.transpose(0, 1, 3, 2))
    Vr = _round_fp32r(V)

    in_maps = []
    for core_units in per_core:
        qt = np.empty((len(core_units), P, S), np.float32)
        kt = np.empty((len(core_units), P, S), np.float32)
        vx = np.zeros((len(core_units), S, P), np.float32)
        for s, (b, h) in enumerate(core_units):
            qt[s, 0:D] = QT[b, h]
            qt[s, D:P] = QT[b, h]
            kt[s, 0:D] = KT[b, h]
            kt[s, D:P] = KT[b, h]
            vx[s, :, 0:D] = Vr[b, h]
            vx[s, :, D] = 1.0
        in_maps.append({"qt": qt, "kt": kt, "vx": vx})
    return out, per_core, in_maps


def _run_device(n_units, in_maps, trace=False, trace_cores=None):
    from concourse import bass_utils

    nc = _get_program(n_units)
    return bass_utils.run_bass_kernel_spmd(
        nc,
        in_maps,
        list(range(NCORES)),
        trace=trace,
        trace_cores=trace_cores,
    )


def kernel(Q, K, V, mask, _trace=False, _result_box=None):
    out, per_core, in_maps = _prepare(Q, K, V, mask)
    if in_maps is None:
        return out
    res = _run_device(len(per_core[0]), in_maps, trace=_trace)
    if _result_box is not None:
        _result_box.append(res)
    for i, core_units in enumerate(per_core):
        core_out = res.results[i]["out"]                  # [n, D, S]
        for s, (b, h) in enumerate(core_units):
            out[b, h] = core_out[s].T
    return out


# revision 13
# speedup vs baseline: 1.7734x; 1.7734x over previous
"""Trainium2 Bass kernel for batched multi-head attention with per-batch mask.

Reference semantics (B=8, H=16, S=1024, D=64):
    scores = Q @ K^T                      # [B,H,S,S]
    scores = where(mask[b], -1e9, scores) # per-batch mask
    attn   = softmax(scores / sqrt(D))
    out    = attn @ V

Key observations used here:
  * A masked batch (mask[b]=True) has constant scores, so softmax is exactly
    uniform (1/S) and out[b,h,q,:] = mean_k V[b,h,k,:].  That degenerate case
    is computed directly on host; only unmasked (b,h) units go to the device.
  * For unmasked units |scores/8| <= ~7, so softmax without max-subtraction
    (exp(x)/sum exp(x)) is numerically safe and mathematically identical.
  * Unmasked units are embarrassingly parallel; they are balanced across the
    8 NeuronCores (H=16 and B=8 make the unit count divisible by 8).

Device algorithm per unit (S=1024 split into 8 chunks of 128 along k):
  mm1:  T[k,q]  = sum_d K[k,d]*Q[q,d]       (lhsT = K^T chunk, rhs = Q^T)
        K=64 matmuls run at half rate, so chunk pairs are packed into row
        groups (0,0)/(64,0) with Q^T/K^T replicated on partitions 64..127 —
        the two half-array matmuls execute concurrently.
  exp:  E[k,q]  = exp(T/8)                  (ScalarE, PSUM->SBUF, fp32r out)
  mm2:  U[m,q] += sum_k Vx[k,m]*E[k,q]      (lhsT = Vx chunk = [V | ones | 0])
        Vx is zero-padded to 128 columns so M=128 stays on the fast path.
        -> U[0:64,q] = unnormalized out^T, U[64,q] = softmax denominator
  out:  recip denominator row, DMA-broadcast it across partitions, one DVE
        multiply, and write out^T [64, S]; the host transposes back.
"""

import numpy as np

B, H, S, D = 8, 16, 1024, 64
P = 128                      # SBUF partitions / k-chunk size
NCHUNK = S // P              # 8 k-chunks per unit
NHALF = 2                    # matmul moving operand is limited to N=512 fp32
NCORES = 8

_program_cache = {}


def _build_program(n_units):
    import concourse.bass as bass
    import concourse.mybir as mybir
    import concourse.tile as tile
    from concourse import bacc

    f32 = mybir.dt.float32
    f32r = mybir.dt.float32r
    nc = bacc.Bacc("TRN2", target_bir_lowering=False, debug=False)

    # qt/kt carry Q^T/K^T duplicated on partitions 64..127 (row-group packing)
    qt_d = nc.dram_tensor("qt", [n_units, P, S], f32r, kind="ExternalInput").ap()
    kt_d = nc.dram_tensor("kt", [n_units, P, S], f32r, kind="ExternalInput").ap()
    vx_d = nc.dram_tensor("vx", [n_units, S, P], f32r, kind="ExternalInput").ap()
    out_d = nc.dram_tensor("out", [n_units, D, S], f32, kind="ExternalOutput").ap()

    with tile.TileContext(nc) as tc:
        with (
            tc.tile_pool(name="qp", bufs=2) as qp,
            tc.tile_pool(name="kp", bufs=2) as kp,
            tc.tile_pool(name="vp", bufs=2) as vp,
            tc.tile_pool(name="ep", bufs=3) as ep,
            tc.tile_pool(name="rp", bufs=2) as rp,
            tc.tile_pool(name="bp", bufs=2) as bp,
            tc.tile_pool(name="op", bufs=2) as op,
            tc.tile_pool(name="pt", bufs=3, space="PSUM") as pt,   # 3 x 2 banks
            tc.tile_pool(name="pu", bufs=1, space="PSUM") as pu,   # 2 banks
        ):
            for j in range(n_units):
                qt = qp.tile([P, S], f32r)
                nc.sync.dma_start(qt, qt_d[j])
                kt = kp.tile([P, S], f32r)
                nc.sync.dma_start(kt, kt_d[j])
                vx = vp.tile([P, NCHUNK, P], f32r)
                nc.sync.dma_start(vx, vx_d[j].rearrange("(c p) d -> p c d", p=P))

                u_ps = pu.tile([P, S], f32)

                def mm1_pair(cp, qt=qt, kt=kt):
                    ca, cb = 2 * cp, 2 * cp + 1
                    ta = pt.tile([P, S], f32, tag="tps", name=f"ta{cp}")
                    tb = pt.tile([P, S], f32, tag="tps", name=f"tb{cp}")
                    for h in range(NHALF):
                        qs = slice(h * 512, (h + 1) * 512)
                        nc.tensor.matmul(
                            ta[:, qs],
                            lhsT=kt[0:D, ca * P:(ca + 1) * P],
                            rhs=qt[0:D, qs],
                            start=True, stop=True,
                            tile_position=(0, 0),
                        )
                        nc.tensor.matmul(
                            tb[:, qs],
                            lhsT=kt[D:P, cb * P:(cb + 1) * P],
                            rhs=qt[D:P, qs],
                            start=True, stop=True,
                            tile_position=(64, 0),
                        )
                    return ta, tb

                # software-pipelined chunk loop: the next pair's mm1s are
                # emitted (adjacently, for row-group packing) before this
                # pair's exp+mm2 consumers.
                NP = NCHUNK // 2
                tiles = mm1_pair(0)
                for cp in range(NP):
                    nxt = mm1_pair(cp + 1) if cp + 1 < NP else None
                    for c, t_ps in zip((2 * cp, 2 * cp + 1), tiles):
                        e_sb = ep.tile([P, S], f32r)
                        nc.scalar.activation(
                            e_sb, t_ps, mybir.ActivationFunctionType.Exp,
                            bias=0.0, scale=0.125,
                        )
                        for h in range(NHALF):
                            qs = slice(h * 512, (h + 1) * 512)
                            nc.tensor.matmul(
                                u_ps[:, qs],
                                lhsT=vx[:, c, :],
                                rhs=e_sb[:, qs],
                                start=(c == 0),
                                stop=(c == NCHUNK - 1),
                            )
                    tiles = nxt

                # U rows 0..63 hold out^T, rows 64..127 hold the softmax
                # denominator (replicated by the ones columns of Vx).  Copy
                # U out of PSUM immediately (releases u_ps for the next
                # unit), then normalize fully in SBUF.
                u_sb = rp.tile([P, S], f32)
                nc.vector.tensor_copy(out=u_sb, in_=u_ps)
                rb = rp.tile([P, S], f32, name="rb")
                nc.vector.reciprocal(out=rb[D:P, :], in_=u_sb[D:P, :])
                r_bc = bp.tile([D, S], f32)
                nc.sync.dma_start(r_bc, rb[D:P, :])
                o_sb = op.tile([D, S], f32)
                nc.vector.tensor_mul(out=o_sb, in0=u_sb[0:D, :], in1=r_bc)
                nc.sync.dma_start(out_d[j], o_sb)
    nc.compile()
    return nc


def _get_program(n_units):
    if n_units not in _program_cache:
        _program_cache[n_units] = _build_program(n_units)
    return _program_cache[n_units]


def _round_fp32r(x):
    """Round fp32 to the fp32r-representable set (bf16 hi + bf16 lo pair).

    The walrus verifier requires fp32r matmul operands to be pre-rounded;
    the PE's replicated fp32 path decomposes each value into two bf16s.
    """
    import ml_dtypes

    hi = x.astype(ml_dtypes.bfloat16).astype(np.float32)
    lo = (x - hi).astype(ml_dtypes.bfloat16).astype(np.float32)
    return hi + lo


def _prepare(Q, K, V, mask):
    """Host-side sharding. Returns (out_skeleton, units_per_core, in_maps)."""
    Q = np.ascontiguousarray(Q, dtype=np.float32)
    K = np.ascontiguousarray(K, dtype=np.float32)
    V = np.ascontiguousarray(V, dtype=np.float32)
    mask_b = np.asarray(mask).reshape(B).astype(bool)

    out = np.empty((B, H, S, D), dtype=np.float32)

    # Masked batches: softmax over a constant row is exactly uniform -> mean of V.
    for b in np.nonzero(mask_b)[0]:
        mv = V[b].mean(axis=1, dtype=np.float32)          # [H, D]
        out[b] = np.broadcast_to(mv[:, None, :], (H, S, D))

    units = [(b, h) for b in range(B) if not mask_b[b] for h in range(H)]
    if not units:
        return out, None, None

    # Pad to a multiple of NCORES with duplicates (identical redundant work).
    n_per = -(-len(units) // NCORES)
    padded = units + [units[0]] * (n_per * NCORES - len(units))
    per_core = [padded[i::NCORES] for i in range(NCORES)]

    QT = _round_fp32r(Q.transpose(0, 1, 3, 2))            # [B,H,D,S]
    KT = _round_fp32r(K.transpose(0, 1, 3, 2))
    Vr = _round_fp32r(V)

    in_maps = []
    for core_units in per_core:
        qt = np.empty((len(core_units), P, S), np.float32)
        kt = np.empty((len(core_units), P, S), np.float32)
        vx = np.zeros((len(core_units), S, P), np.float32)
        for s, (b, h) in enumerate(core_units):
            qt[s, 0:D] = QT[b, h]
            qt[s, D:P] = QT[b, h]
            kt[s, 0:D] = KT[b, h]
            kt[s, D:P] = KT[b, h]
            vx[s, :, 0:D] = Vr[b, h]
            vx[s, :, D:P] = 1.0
        in_maps.append({"qt": qt, "kt": kt, "vx": vx})
    return out, per_core, in_maps


def _run_device(n_units, in_maps, trace=False, trace_cores=None):
    from concourse import bass_utils

    nc = _get_program(n_units)
    return bass_utils.run_bass_kernel_spmd(
        nc,
        in_maps,
        list(range(NCORES)),
        trace=trace,
        trace_cores=trace_cores,
    )


def kernel(Q, K, V, mask, _trace=False, _result_box=None):
    out, per_core, in_maps = _prepare(Q, K, V, mask)
    if in_maps is None:
        return out
    res = _run_device(len(per_core[0]), in_maps, trace=_trace)
    if _result_box is not None:
        _result_box.append(res)
    for i, core_units in enumerate(per_core):
        core_out = res.results[i]["out"]                  # [n, D, S]
        for s, (b, h) in enumerate(core_units):
            out[b, h] = core_out[s].T
    return out
